# revision 11
# baseline (speedup 1.0000x reference)
"""Trainium2 Bass kernel for NetworksPlusCircuit.

Two MLPs (784->512->10, relu, softmax) over B=65536 samples each, then
P(sum=s) = sum_{a+c=s} p0[a]*p1[c]  -> [B, 19].

Sharding: pure data parallel over the batch across 8 NeuronCores.
Host-side prep: shard + transpose x to [784, B/8] per core so the device
DMA streams are contiguous; weights replicated (cast to bf16 on host).

Device-side per core (BC = 8192 samples per net):
  per round (512 samples of one net):
    - SWDGE cast-DMA loads xT tile [128f, 7fc, 512b] (fp32 HBM -> bf16 SBUF)
    - mm1: hT[j, b] += W1[f, j].T @ xT[f, b]   (PSUM, 4 j-chunks x 7 f-chunks)
    - bias+relu+cast -> SBUF bf16 (split across DVE and ACT)
    - mm2: l[b, 10]  += hT[j, b].T @ W2[j, 10] (PSUM, accumulate 4 j-chunks)
    - exp (ACT, PSUM -> SBUF block buffer), softmax denominator deferred
  per block (8 pair-rounds = 4096 sample pairs):
    - scale by exp(b2), Z sums, digit-sum convolution, normalize (DVE), DMA out
"""

import sys
import types

import numpy as np
import concourse.bass as bass
import concourse.bacc as bacc
import concourse.mybir as mybir
import concourse.tile as tile
from concourse.bass_utils import run_bass_kernel_spmd

F32 = mybir.dt.float32
MMDT = mybir.dt.float16  # matmul dtype: fp16 = bf16 speed, 4x less rounding error

NCORES = 8
B = 65536
BC = B // NCORES            # samples per core per net
F = 784                     # input features
HID = 512
NCLS = 10
NSUM = 19
FCH_FULL = F // 128         # 6 full feature chunks
FT = F - FCH_FULL * 128     # 16 tail features
FCH = FCH_FULL + 1          # 7 chunks
JCH = HID // 128            # 4 hidden chunks
RN = 512                    # samples per round
NR = BC // RN               # 16 pair-rounds per core
NBLK = 2                    # conv blocks; NR % NBLK == 0
RPB = NR // NBLK            # pair-rounds per block (8)
GPB = RPB * (RN // 128)     # 128-sample groups per block (32)


def build_nc():
    nc = bacc.Bacc("TRN2", target_bir_lowering=False, debug=False, num_devices=NCORES)

    xt = [
        nc.dram_tensor(f"xt{n}", [F, BC], F32, kind="ExternalInput") for n in range(2)
    ]
    w1 = [
        nc.dram_tensor(f"w1_{n}", [F, HID], MMDT, kind="ExternalInput")
        for n in range(2)
    ]
    w2 = [
        nc.dram_tensor(f"w2_{n}", [HID, NCLS], MMDT, kind="ExternalInput")
        for n in range(2)
    ]
    b1 = [
        nc.dram_tensor(f"b1_{n}", [HID], F32, kind="ExternalInput") for n in range(2)
    ]
    eb2 = [
        nc.dram_tensor(f"eb2_{n}", [NCLS], F32, kind="ExternalInput") for n in range(2)
    ]
    out = nc.dram_tensor("out", [BC, NSUM], F32, kind="ExternalOutput")

    with tile.TileContext(nc) as tc:
        with (
            tc.tile_pool(name="consts", bufs=1) as consts,
            tc.tile_pool(name="xt", bufs=3) as xt_pool,
            tc.tile_pool(name="ht", bufs=2) as ht_pool,
            tc.tile_pool(name="eblk", bufs=2) as e_pool,
            tc.tile_pool(name="conv", bufs=1) as conv_pool,
            tc.tile_pool(name="outp", bufs=2) as out_pool,
            tc.tile_pool(name="hpsum", bufs=6, space="PSUM") as h_pool,
            tc.tile_pool(name="lpsum", bufs=2, space="PSUM") as l_pool,
        ):
            # ---- constants ----
            w1_sb = []
            w2_sb = []
            b1_sb = []
            eb2_sb = []
            for n in range(2):
                w = consts.tile([128, FCH, HID], MMDT, name=f"w1sb{n}", tag=f"w1sb{n}")
                nc.gpsimd.dma_start(
                    out=w[:, 0:FCH_FULL, :],
                    in_=w1[n].ap()[0 : FCH_FULL * 128, :].rearrange(
                        "(c p) j -> p c j", p=128
                    ),
                )
                nc.gpsimd.dma_start(
                    out=w[0:FT, FCH_FULL, :], in_=w1[n].ap()[FCH_FULL * 128 : F, :]
                )
                w1_sb.append(w)

                w2t = consts.tile([128, JCH, NCLS], MMDT, name=f"w2sb{n}", tag=f"w2sb{n}")
                nc.gpsimd.dma_start(
                    out=w2t, in_=w2[n].ap().rearrange("(c p) a -> p c a", p=128)
                )
                w2_sb.append(w2t)

                b1t = consts.tile([128, JCH], F32, name=f"b1sb{n}", tag=f"b1sb{n}")
                nc.gpsimd.dma_start(
                    out=b1t, in_=b1[n].ap().rearrange("(c p) -> p c", p=128)
                )
                b1_sb.append(b1t)

                ebt = consts.tile([128, NCLS], F32, name=f"eb2sb{n}", tag=f"eb2sb{n}")
                src = eb2[n].ap()
                nc.gpsimd.dma_start(
                    out=ebt,
                    in_=bass.AP(tensor=src.tensor, offset=0, ap=[[0, 128], [1, NCLS]]),
                )
                eb2_sb.append(ebt)

            # ---- main loop ----
            for blk in range(NBLK):
                # exp values for this block: [128, net, group, cls]
                e_blk = e_pool.tile([128, 2, GPB, NCLS], F32, name="eblk", tag="eblk")

                for rr in range(RPB):
                    r = blk * RPB + rr
                    for n in range(2):
                        xtile = xt_pool.tile([128, FCH, RN], MMDT, name="xt", tag="xt")
                        src = xt[n].ap()[:, r * RN : (r + 1) * RN]
                        nc.gpsimd.dma_start(
                            out=xtile[:, 0:FCH_FULL, :],
                            in_=src[0 : FCH_FULL * 128, :].rearrange(
                                "(c p) b -> p c b", p=128
                            ),
                        )
                        nc.gpsimd.dma_start(
                            out=xtile[0:FT, FCH_FULL, :],
                            in_=src[FCH_FULL * 128 : F, :],
                        )

                        ph = [
                            h_pool.tile([128, RN], F32, name="ph", tag="ph")
                            for i in range(JCH)
                        ]
                        for jc in range(JCH):
                            for fc in range(FCH):
                                kk = 128 if fc < FCH_FULL else FT
                                nc.tensor.matmul(
                                    out=ph[jc][:, :],
                                    lhsT=w1_sb[n][0:kk, fc, jc * 128 : (jc + 1) * 128],
                                    rhs=xtile[0:kk, fc, :],
                                    start=(fc == 0),
                                    stop=(fc == FCH - 1),
                                )

                        ht = ht_pool.tile([128, JCH, RN], MMDT, name="ht", tag="ht")
                        for jc in range(JCH):
                            if jc % 2 == 0:
                                nc.vector.tensor_scalar(
                                    out=ht[:, jc, :],
                                    in0=ph[jc][:, :],
                                    scalar1=b1_sb[n][:, jc : jc + 1],
                                    scalar2=0.0,
                                    op0=mybir.AluOpType.add,
                                    op1=mybir.AluOpType.max,
                                )
                            else:
                                nc.scalar.activation(
                                    out=ht[:, jc, :],
                                    in_=ph[jc][:, :],
                                    func=mybir.ActivationFunctionType.Relu,
                                    bias=b1_sb[n][:, jc : jc + 1],
                                    scale=1.0,
                                )

                        pl = l_pool.tile([128, RN // 128, NCLS], F32, name="pl", tag="pl")
                        for bc in range(RN // 128):
                            for jc in range(JCH):
                                nc.tensor.matmul(
                                    out=pl[:, bc, :],
                                    lhsT=ht[:, jc, bc * 128 : (bc + 1) * 128],
                                    rhs=w2_sb[n][:, jc, :],
                                    start=(jc == 0),
                                    stop=(jc == JCH - 1),
                                )

                        nc.scalar.activation(
                            out=e_blk[:, n, rr * 4 : (rr + 1) * 4, :],
                            in_=pl[:, :, :],
                            func=mybir.ActivationFunctionType.Exp,
                        )

                # ---- conv block over GPB groups ----
                # scale by exp(b2) (free-broadcast over groups)
                for n in range(2):
                    nc.vector.tensor_tensor(
                        out=e_blk[:, n, :, :],
                        in0=e_blk[:, n, :, :],
                        in1=eb2_sb[n][:, None, :].to_broadcast([128, GPB, NCLS]),
                        op=mybir.AluOpType.mult,
                    )

                z = conv_pool.tile([128, 2, GPB], F32, name="z", tag="z")
                for n in range(2):
                    nc.vector.reduce_sum(
                        out=z[:, n, :],
                        in_=e_blk[:, n, :, :],
                        axis=mybir.AxisListType.X,
                    )
                rz = conv_pool.tile([128, GPB], F32, name="rz", tag="rz")
                nc.vector.tensor_tensor(
                    out=rz, in0=z[:, 0, :], in1=z[:, 1, :], op=mybir.AluOpType.mult
                )
                nc.vector.reciprocal(out=rz, in_=rz)

                prods = conv_pool.tile([128, GPB, NCLS, NCLS], F32, name="prods", tag="prods")
                for a in range(NCLS):
                    nc.vector.tensor_tensor(
                        out=prods[:, :, a, :],
                        in0=e_blk[:, 1, :, :],
                        in1=e_blk[:, 0, :, a : a + 1].to_broadcast(
                            [128, GPB, NCLS]
                        ),
                        op=mybir.AluOpType.mult,
                    )

                acc = out_pool.tile([128, GPB, NSUM], F32, name="acc", tag="acc")
                nc.vector.memset(acc, 0.0)
                for a in range(NCLS):
                    nc.vector.tensor_tensor(
                        out=acc[:, :, a : a + NCLS],
                        in0=acc[:, :, a : a + NCLS],
                        in1=prods[:, :, a, :],
                        op=mybir.AluOpType.add,
                    )
                nc.vector.tensor_tensor(
                    out=acc,
                    in0=acc,
                    in1=rz[:, :, None].to_broadcast([128, GPB, NSUM]),
                    op=mybir.AluOpType.mult,
                )
                nc.sync.dma_start(
                    out=out.ap()[blk * GPB * 128 : (blk + 1) * GPB * 128, :].rearrange(
                        "(g p) s -> p g s", p=128
                    ),
                    in_=acc,
                )

    nc.compile()
    return nc


_NC_CACHE = {}


def _get_nc():
    if "nc" not in _NC_CACHE:
        _NC_CACHE["nc"] = build_nc()
    return _NC_CACHE["nc"]


def _install_ntff_hook():
    """Shim antenv.axon_hooks (absent in this image) so trace=True can work."""
    try:
        import antenv

        if hasattr(antenv, "axon_hooks"):
            return
        from trn_agent_boot.trn_boot import _ntff_profile_via_ctypes

        mod = types.ModuleType("antenv.axon_hooks")
        holder = {"hook": _ntff_profile_via_ctypes("/opt/axon/libaxon_pjrt.so")}
        mod.set_axon_ntff_profile_hook = lambda h: holder.__setitem__("hook", h)
        mod.get_axon_ntff_profile_hook = lambda: holder["hook"]
        sys.modules["antenv.axon_hooks"] = mod
        antenv.axon_hooks = mod
    except Exception:
        pass


def kernel(x, W1_0, b1_0, W2_0, b2_0, W1_1, b1_1, W2_1, b2_1, _trace=False):
    x = np.asarray(x, dtype=np.float32)

    xf = x.reshape(2, B, F)
    weights = {}
    for n, (W1n, b1n, W2n, b2n) in enumerate(
        [(W1_0, b1_0, W2_0, b2_0), (W1_1, b1_1, W2_1, b2_1)]
    ):
        weights[f"w1_{n}"] = np.ascontiguousarray(np.asarray(W1n, np.float32)).astype(np.float16)
        weights[f"w2_{n}"] = np.ascontiguousarray(np.asarray(W2n, np.float32)).astype(np.float16)
        weights[f"b1_{n}"] = np.ascontiguousarray(np.asarray(b1n, np.float32))
        weights[f"eb2_{n}"] = np.exp(np.asarray(b2n, np.float32))

    in_maps = []
    for c in range(NCORES):
        m = dict(weights)
        for n in range(2):
            shard = xf[n, c * BC : (c + 1) * BC, :]  # [BC, F]
            m[f"xt{n}"] = np.ascontiguousarray(shard.T)  # [F, BC]
        in_maps.append(m)

    nc = _get_nc()
    if _trace:
        _install_ntff_hook()
    res = run_bass_kernel_spmd(nc, in_maps, list(range(NCORES)), trace=_trace)
    pieces = [res.results[c]["out"] for c in range(NCORES)]
    full = np.concatenate(pieces, axis=0).astype(np.float32)
    if _trace:
        return full, res
    return full


# revision 17
# speedup vs baseline: 1.0395x; 1.0395x over previous
"""Trainium2 Bass kernel for NetworksPlusCircuit.

Two MLPs (784->512->10, relu, softmax) over B=65536 samples each, then
P(sum=s) = sum_{a+c=s} p0[a]*p1[c]  -> [B, 19].

Sharding: pure data parallel over the batch across 8 NeuronCores.
Host-side prep: shard + transpose x to [784, B/8] per core so the device
DMA streams are contiguous; weights replicated (cast to bf16 on host).

Device-side per core (BC = 8192 samples per net):
  per round (512 samples of one net):
    - SWDGE cast-DMA loads xT tile [128f, 7fc, 512b] (fp32 HBM -> bf16 SBUF)
    - mm1: hT[j, b] += W1[f, j].T @ xT[f, b]   (PSUM, 4 j-chunks x 7 f-chunks)
    - bias+relu+cast -> SBUF bf16 (split across DVE and ACT)
    - mm2: l[b, 10]  += hT[j, b].T @ W2[j, 10] (PSUM, accumulate 4 j-chunks)
    - exp (ACT, PSUM -> SBUF block buffer), softmax denominator deferred
  per block (8 pair-rounds = 4096 sample pairs):
    - scale by exp(b2), Z sums, digit-sum convolution, normalize (DVE), DMA out
"""

import sys
import types

import numpy as np
import concourse.bass as bass
import concourse.bacc as bacc
import concourse.mybir as mybir
import concourse.tile as tile
from concourse.bass_utils import run_bass_kernel_spmd

F32 = mybir.dt.float32
MMDT = mybir.dt.float16  # matmul dtype: fp16 = bf16 speed, 4x less rounding error

NCORES = 8
B = 65536
BC = B // NCORES            # samples per core per net
F = 784                     # input features
HID = 512
NCLS = 10
NSUM = 19
FCH_FULL = F // 128         # 6 full feature chunks
FT = F - FCH_FULL * 128     # 16 tail features
FCH = FCH_FULL + 1          # 7 chunks
JCH = HID // 128            # 4 hidden chunks
# The 16 tail features are replicated at 4 32-row strips (rows 768+32k..+16)
# so the 4 per-j-chunk tail matmuls sit in distinct PE row-groups and run
# concurrently (row tiling).
FPAD = FCH * 128  # padded rows: 6*128 full chunks + 4 strips * 32 = 896
RN = 512                    # samples per round
NR = BC // RN               # 16 pair-rounds per core
NBLK = 4                    # conv blocks; NR % NBLK == 0
RPB = NR // NBLK            # pair-rounds per block
GPB = RPB * (RN // 128)     # 128-sample groups per block


def pad_rows(a, dtype):
    """[F, ...] -> [FPAD, ...] with tail rows replicated at 4 32-row strips."""
    out = np.zeros((FPAD,) + a.shape[1:], dtype=dtype)
    out[: FCH_FULL * 128] = a[: FCH_FULL * 128]
    for k in range(JCH):
        base = FCH_FULL * 128 + 32 * k
        out[base : base + FT] = a[FCH_FULL * 128 :]
    return out


def build_nc():
    nc = bacc.Bacc("TRN2", target_bir_lowering=False, debug=False, num_devices=NCORES)

    xt = [
        nc.dram_tensor(f"xt{n}", [FPAD, BC], F32, kind="ExternalInput")
        for n in range(2)
    ]
    w1 = [
        nc.dram_tensor(f"w1_{n}", [FPAD, HID], MMDT, kind="ExternalInput")
        for n in range(2)
    ]
    w2 = [
        nc.dram_tensor(f"w2_{n}", [HID, NCLS], MMDT, kind="ExternalInput")
        for n in range(2)
    ]
    b1 = [
        nc.dram_tensor(f"b1_{n}", [HID], F32, kind="ExternalInput") for n in range(2)
    ]
    eb2 = [
        nc.dram_tensor(f"eb2_{n}", [NCLS], F32, kind="ExternalInput") for n in range(2)
    ]
    out = nc.dram_tensor("out", [BC, NSUM], F32, kind="ExternalOutput")

    with tile.TileContext(nc) as tc:
        with (
            tc.tile_pool(name="consts", bufs=1) as consts,
            tc.tile_pool(name="xt", bufs=3) as xt_pool,
            tc.tile_pool(name="ht", bufs=2) as ht_pool,
            tc.tile_pool(name="eblk", bufs=2) as e_pool,
            tc.tile_pool(name="conv", bufs=1) as conv_pool,
            tc.tile_pool(name="outp", bufs=2) as out_pool,
            tc.tile_pool(name="hpsum", bufs=6, space="PSUM") as h_pool,
            tc.tile_pool(name="lpsum", bufs=2, space="PSUM") as l_pool,
        ):
            # ---- constants ----
            w1_sb = []
            w2_sb = []
            b1_sb = []
            eb2_sb = []
            for n in range(2):
                w = consts.tile([128, FCH, HID], MMDT, name=f"w1sb{n}", tag=f"w1sb{n}")
                nc.sync.dma_start(
                    out=w, in_=w1[n].ap().rearrange("(c p) j -> p c j", p=128)
                )
                w1_sb.append(w)

                w2t = consts.tile([128, JCH, NCLS], MMDT, name=f"w2sb{n}", tag=f"w2sb{n}")
                nc.sync.dma_start(
                    out=w2t, in_=w2[n].ap().rearrange("(c p) a -> p c a", p=128)
                )
                w2_sb.append(w2t)

                b1t = consts.tile([128, JCH], F32, name=f"b1sb{n}", tag=f"b1sb{n}")
                nc.gpsimd.dma_start(
                    out=b1t, in_=b1[n].ap().rearrange("(c p) -> p c", p=128)
                )
                b1_sb.append(b1t)

                ebt = consts.tile([128, NCLS], F32, name=f"eb2sb{n}", tag=f"eb2sb{n}")
                src = eb2[n].ap()
                nc.gpsimd.dma_start(
                    out=ebt,
                    in_=bass.AP(tensor=src.tensor, offset=0, ap=[[0, 128], [1, NCLS]]),
                )
                eb2_sb.append(ebt)

            # ---- main loop ----
            for blk in range(NBLK):
                # exp values for this block: [128, net, group, cls]
                e_blk = e_pool.tile([128, 2, GPB, NCLS], F32, name="eblk", tag="eblk")

                for rr in range(RPB):
                    r = blk * RPB + rr
                    for n in range(2):
                        xtile = xt_pool.tile([128, FCH, RN], MMDT, name="xt", tag="xt")
                        src = xt[n].ap()[:, r * RN : (r + 1) * RN]
                        nc.gpsimd.dma_start(
                            out=xtile,
                            in_=src.rearrange("(c p) b -> p c b", p=128),
                        )

                        ph = [
                            h_pool.tile([128, RN], F32, name="ph", tag="ph")
                            for i in range(JCH)
                        ]
                        for jc in range(JCH):
                            for fc in range(FCH_FULL):
                                nc.tensor.matmul(
                                    out=ph[jc][:, :],
                                    lhsT=w1_sb[n][:, fc, jc * 128 : (jc + 1) * 128],
                                    rhs=xtile[:, fc, :],
                                    start=(fc == 0),
                                    stop=False,
                                )
                        # tail features: 4 concurrent row-tiled K=16 matmuls
                        for jc in range(JCH):
                            p0 = 32 * jc
                            nc.tensor.matmul(
                                out=ph[jc][:, :],
                                lhsT=w1_sb[n][
                                    p0 : p0 + FT, FCH_FULL, jc * 128 : (jc + 1) * 128
                                ],
                                rhs=xtile[p0 : p0 + FT, FCH_FULL, :],
                                start=False,
                                stop=True,
                                tile_position=(p0, 0),
                            )

                        ht = ht_pool.tile([128, JCH, RN], MMDT, name="ht", tag="ht")
                        for jc in range(JCH):
                            if jc % 2 == 0:
                                nc.vector.tensor_scalar(
                                    out=ht[:, jc, :],
                                    in0=ph[jc][:, :],
                                    scalar1=b1_sb[n][:, jc : jc + 1],
                                    scalar2=0.0,
                                    op0=mybir.AluOpType.add,
                                    op1=mybir.AluOpType.max,
                                )
                            else:
                                nc.scalar.activation(
                                    out=ht[:, jc, :],
                                    in_=ph[jc][:, :],
                                    func=mybir.ActivationFunctionType.Relu,
                                    bias=b1_sb[n][:, jc : jc + 1],
                                    scale=1.0,
                                )

                        pl = l_pool.tile([128, RN // 128, NCLS], F32, name="pl", tag="pl")
                        for bc in range(RN // 128):
                            for jc in range(JCH):
                                nc.tensor.matmul(
                                    out=pl[:, bc, :],
                                    lhsT=ht[:, jc, bc * 128 : (bc + 1) * 128],
                                    rhs=w2_sb[n][:, jc, :],
                                    start=(jc == 0),
                                    stop=(jc == JCH - 1),
                                )

                        nc.scalar.activation(
                            out=e_blk[:, n, rr * 4 : (rr + 1) * 4, :],
                            in_=pl[:, :, :],
                            func=mybir.ActivationFunctionType.Exp,
                        )

                # ---- conv block over GPB groups ----
                # scale by exp(b2) (free-broadcast over groups)
                for n in range(2):
                    nc.vector.tensor_tensor(
                        out=e_blk[:, n, :, :],
                        in0=e_blk[:, n, :, :],
                        in1=eb2_sb[n][:, None, :].to_broadcast([128, GPB, NCLS]),
                        op=mybir.AluOpType.mult,
                    )

                z = conv_pool.tile([128, 2, GPB], F32, name="z", tag="z")
                for n in range(2):
                    nc.vector.reduce_sum(
                        out=z[:, n, :],
                        in_=e_blk[:, n, :, :],
                        axis=mybir.AxisListType.X,
                    )
                rz = conv_pool.tile([128, GPB], F32, name="rz", tag="rz")
                nc.vector.tensor_tensor(
                    out=rz, in0=z[:, 0, :], in1=z[:, 1, :], op=mybir.AluOpType.mult
                )
                nc.vector.reciprocal(out=rz, in_=rz)

                prods = conv_pool.tile([128, GPB, NCLS, NCLS], F32, name="prods", tag="prods")
                for a in range(NCLS):
                    nc.vector.tensor_tensor(
                        out=prods[:, :, a, :],
                        in0=e_blk[:, 1, :, :],
                        in1=e_blk[:, 0, :, a : a + 1].to_broadcast(
                            [128, GPB, NCLS]
                        ),
                        op=mybir.AluOpType.mult,
                    )

                acc = out_pool.tile([128, GPB, NSUM], F32, name="acc", tag="acc")
                nc.vector.memset(acc, 0.0)
                for a in range(NCLS):
                    nc.vector.tensor_tensor(
                        out=acc[:, :, a : a + NCLS],
                        in0=acc[:, :, a : a + NCLS],
                        in1=prods[:, :, a, :],
                        op=mybir.AluOpType.add,
                    )
                nc.vector.tensor_tensor(
                    out=acc,
                    in0=acc,
                    in1=rz[:, :, None].to_broadcast([128, GPB, NSUM]),
                    op=mybir.AluOpType.mult,
                )
                nc.sync.dma_start(
                    out=out.ap()[blk * GPB * 128 : (blk + 1) * GPB * 128, :].rearrange(
                        "(g p) s -> p g s", p=128
                    ),
                    in_=acc,
                )

    nc.compile()
    return nc


_NC_CACHE = {}


def _get_nc():
    if "nc" not in _NC_CACHE:
        _NC_CACHE["nc"] = build_nc()
    return _NC_CACHE["nc"]


def _install_ntff_hook():
    """Shim antenv.axon_hooks (absent in this image) so trace=True can work."""
    try:
        import antenv

        if hasattr(antenv, "axon_hooks"):
            return
        from trn_agent_boot.trn_boot import _ntff_profile_via_ctypes

        mod = types.ModuleType("antenv.axon_hooks")
        holder = {"hook": _ntff_profile_via_ctypes("/opt/axon/libaxon_pjrt.so")}
        mod.set_axon_ntff_profile_hook = lambda h: holder.__setitem__("hook", h)
        mod.get_axon_ntff_profile_hook = lambda: holder["hook"]
        sys.modules["antenv.axon_hooks"] = mod
        antenv.axon_hooks = mod
    except Exception:
        pass


def kernel(x, W1_0, b1_0, W2_0, b2_0, W1_1, b1_1, W2_1, b2_1, _trace=False):
    x = np.asarray(x, dtype=np.float32)

    xf = x.reshape(2, B, F)
    weights = {}
    for n, (W1n, b1n, W2n, b2n) in enumerate(
        [(W1_0, b1_0, W2_0, b2_0), (W1_1, b1_1, W2_1, b2_1)]
    ):
        weights[f"w1_{n}"] = pad_rows(
            np.asarray(W1n, np.float32).astype(np.float16), np.float16
        )
        weights[f"w2_{n}"] = np.ascontiguousarray(np.asarray(W2n, np.float32)).astype(np.float16)
        weights[f"b1_{n}"] = np.ascontiguousarray(np.asarray(b1n, np.float32))
        weights[f"eb2_{n}"] = np.exp(np.asarray(b2n, np.float32))

    in_maps = []
    for c in range(NCORES):
        m = dict(weights)
        for n in range(2):
            shard = xf[n, c * BC : (c + 1) * BC, :]  # [BC, F]
            m[f"xt{n}"] = pad_rows(np.ascontiguousarray(shard.T), np.float32)
        in_maps.append(m)

    nc = _get_nc()
    if _trace:
        _install_ntff_hook()
    res = run_bass_kernel_spmd(nc, in_maps, list(range(NCORES)), trace=_trace)
    pieces = [res.results[c]["out"] for c in range(NCORES)]
    full = np.concatenate(pieces, axis=0).astype(np.float32)
    if _trace:
        return full, res
    return full


# revision 20
# speedup vs baseline: 1.1219x; 1.0793x over previous
"""Trainium2 Bass kernel for NetworksPlusCircuit.

Two MLPs (784->512->10, relu, softmax) over B=65536 samples each, then
P(sum=s) = sum_{a+c=s} p0[a]*p1[c]  -> [B, 19].

Sharding: pure data parallel over the batch across 8 NeuronCores.
Host-side prep: shard + transpose x to [784, B/8] per core so the device
DMA streams are contiguous; weights replicated (cast to bf16 on host).

Device-side per core (BC = 8192 samples per net):
  per round (512 samples of one net):
    - SWDGE cast-DMA loads xT tile [128f, 7fc, 512b] (fp32 HBM -> bf16 SBUF)
    - mm1: hT[j, b] += W1[f, j].T @ xT[f, b]   (PSUM, 4 j-chunks x 7 f-chunks)
    - bias+relu+cast -> SBUF bf16 (split across DVE and ACT)
    - mm2: l[b, 10]  += hT[j, b].T @ W2[j, 10] (PSUM, accumulate 4 j-chunks)
    - exp (ACT, PSUM -> SBUF block buffer), softmax denominator deferred
  per block (8 pair-rounds = 4096 sample pairs):
    - scale by exp(b2), Z sums, digit-sum convolution, normalize (DVE), DMA out
"""

import sys
import types

import numpy as np
import concourse.bass as bass
import concourse.bacc as bacc
import concourse.mybir as mybir
import concourse.tile as tile
from concourse.bass_utils import run_bass_kernel_spmd

F32 = mybir.dt.float32
MMDT = mybir.dt.float16  # matmul dtype: fp16 = bf16 speed, 4x less rounding error

NCORES = 8
B = 65536
BC = B // NCORES            # samples per core per net
F = 784                     # input features
HID = 512
NCLS = 10
NSUM = 19
FCH_FULL = F // 128         # 6 full feature chunks
FT = F - FCH_FULL * 128     # 16 tail features
FCH = FCH_FULL + 1          # 7 chunks
JCH = HID // 128            # 4 hidden chunks
# The 16 tail features are replicated at 4 32-row strips (rows 768+32k..+16)
# so the 4 per-j-chunk tail matmuls sit in distinct PE row-groups and run
# concurrently (row tiling).
FPAD = FCH * 128  # padded rows: 6*128 full chunks + 4 strips * 32 = 896
RN = 512                    # samples per round
NR = BC // RN               # 16 pair-rounds per core
NBLK = 4                    # conv blocks; NR % NBLK == 0
RPB = NR // NBLK            # pair-rounds per block
GPB = RPB * (RN // 128)     # 128-sample groups per block


def pad_rows(a, dtype):
    """[F, ...] -> [FPAD, ...] with tail rows replicated at 4 32-row strips."""
    out = np.zeros((FPAD,) + a.shape[1:], dtype=dtype)
    out[: FCH_FULL * 128] = a[: FCH_FULL * 128]
    for k in range(JCH):
        base = FCH_FULL * 128 + 32 * k
        out[base : base + FT] = a[FCH_FULL * 128 :]
    return out


def build_nc():
    nc = bacc.Bacc("TRN2", target_bir_lowering=False, debug=False, num_devices=NCORES)

    xt = [
        nc.dram_tensor(f"xt{n}", [FPAD, BC], F32, kind="ExternalInput")
        for n in range(2)
    ]
    w1 = [
        nc.dram_tensor(f"w1_{n}", [FPAD, HID], MMDT, kind="ExternalInput")
        for n in range(2)
    ]
    w2 = [
        nc.dram_tensor(f"w2_{n}", [HID, NCLS], MMDT, kind="ExternalInput")
        for n in range(2)
    ]
    b1 = [
        nc.dram_tensor(f"b1_{n}", [HID], F32, kind="ExternalInput") for n in range(2)
    ]
    eb2 = [
        nc.dram_tensor(f"eb2_{n}", [NCLS], F32, kind="ExternalInput") for n in range(2)
    ]
    out = nc.dram_tensor("out", [BC, NSUM], F32, kind="ExternalOutput")

    with tile.TileContext(nc) as tc:
        with (
            tc.tile_pool(name="consts", bufs=1) as consts,
            tc.tile_pool(name="xt", bufs=3) as xt_pool,
            tc.tile_pool(name="ht", bufs=2) as ht_pool,
            tc.tile_pool(name="eblk", bufs=3) as e_pool,
            tc.tile_pool(name="conv", bufs=1) as conv_pool,
            tc.tile_pool(name="outp", bufs=2) as out_pool,
            tc.tile_pool(name="hpsum", bufs=6, space="PSUM") as h_pool,
            tc.tile_pool(name="lpsum", bufs=2, space="PSUM") as l_pool,
        ):
            # ---- constants ----
            w1_sb = []
            w2_sb = []
            b1_sb = []
            eb2_sb = []
            for n in range(2):
                w = consts.tile([128, FCH, HID], MMDT, name=f"w1sb{n}", tag=f"w1sb{n}")
                nc.sync.dma_start(
                    out=w, in_=w1[n].ap().rearrange("(c p) j -> p c j", p=128)
                )
                w1_sb.append(w)

                w2t = consts.tile([128, JCH, NCLS], MMDT, name=f"w2sb{n}", tag=f"w2sb{n}")
                nc.sync.dma_start(
                    out=w2t, in_=w2[n].ap().rearrange("(c p) a -> p c a", p=128)
                )
                w2_sb.append(w2t)

                b1t = consts.tile([128, JCH], F32, name=f"b1sb{n}", tag=f"b1sb{n}")
                nc.gpsimd.dma_start(
                    out=b1t, in_=b1[n].ap().rearrange("(c p) -> p c", p=128)
                )
                b1_sb.append(b1t)

                ebt = consts.tile([128, NCLS], F32, name=f"eb2sb{n}", tag=f"eb2sb{n}")
                src = eb2[n].ap()
                nc.gpsimd.dma_start(
                    out=ebt,
                    in_=bass.AP(tensor=src.tensor, offset=0, ap=[[0, 128], [1, NCLS]]),
                )
                eb2_sb.append(ebt)

            # ---- main loop ----
            for blk in range(NBLK):
                # exp values for this block: [128, net, group, cls]
                e_blk = e_pool.tile([128, 2, GPB, NCLS], F32, name="eblk", tag="eblk")

                for rr in range(RPB):
                    r = blk * RPB + rr
                    for n in range(2):
                        xtile = xt_pool.tile([128, FCH, RN], MMDT, name="xt", tag="xt")
                        src = xt[n].ap()[:, r * RN : (r + 1) * RN]
                        nc.gpsimd.dma_start(
                            out=xtile,
                            in_=src.rearrange("(c p) b -> p c b", p=128),
                        )

                        ph = [
                            h_pool.tile([128, RN], F32, name="ph", tag="ph")
                            for i in range(JCH)
                        ]
                        for fc in range(FCH_FULL):
                            for jc in range(JCH):
                                nc.tensor.matmul(
                                    out=ph[jc][:, :],
                                    lhsT=w1_sb[n][:, fc, jc * 128 : (jc + 1) * 128],
                                    rhs=xtile[:, fc, :],
                                    start=(fc == 0),
                                    stop=False,
                                )
                        # tail features: 4 concurrent row-tiled K=16 matmuls
                        for jc in range(JCH):
                            p0 = 32 * jc
                            nc.tensor.matmul(
                                out=ph[jc][:, :],
                                lhsT=w1_sb[n][
                                    p0 : p0 + FT, FCH_FULL, jc * 128 : (jc + 1) * 128
                                ],
                                rhs=xtile[p0 : p0 + FT, FCH_FULL, :],
                                start=False,
                                stop=True,
                                tile_position=(p0, 0),
                            )

                        ht = ht_pool.tile([128, JCH, RN], MMDT, name="ht", tag="ht")
                        for jc in range(JCH):
                            # all on ACT: DVE must stay free for the conv
                            # blocks or PSUM drains stall and starve the PE
                            nc.scalar.activation(
                                out=ht[:, jc, :],
                                in_=ph[jc][:, :],
                                func=mybir.ActivationFunctionType.Relu,
                                bias=b1_sb[n][:, jc : jc + 1],
                                scale=1.0,
                            )

                        pl = l_pool.tile([128, RN // 128, NCLS], F32, name="pl", tag="pl")
                        for bc in range(RN // 128):
                            for jc in range(JCH):
                                nc.tensor.matmul(
                                    out=pl[:, bc, :],
                                    lhsT=ht[:, jc, bc * 128 : (bc + 1) * 128],
                                    rhs=w2_sb[n][:, jc, :],
                                    start=(jc == 0),
                                    stop=(jc == JCH - 1),
                                )

                        nc.scalar.activation(
                            out=e_blk[:, n, rr * 4 : (rr + 1) * 4, :],
                            in_=pl[:, :, :],
                            func=mybir.ActivationFunctionType.Exp,
                        )

                # ---- conv block over GPB groups ----
                # scale by exp(b2) (free-broadcast over groups)
                for n in range(2):
                    nc.vector.tensor_tensor(
                        out=e_blk[:, n, :, :],
                        in0=e_blk[:, n, :, :],
                        in1=eb2_sb[n][:, None, :].to_broadcast([128, GPB, NCLS]),
                        op=mybir.AluOpType.mult,
                    )

                z = conv_pool.tile([128, 2, GPB], F32, name="z", tag="z")
                for n in range(2):
                    nc.vector.reduce_sum(
                        out=z[:, n, :],
                        in_=e_blk[:, n, :, :],
                        axis=mybir.AxisListType.X,
                    )
                rz = conv_pool.tile([128, GPB], F32, name="rz", tag="rz")
                nc.vector.tensor_tensor(
                    out=rz, in0=z[:, 0, :], in1=z[:, 1, :], op=mybir.AluOpType.mult
                )
                nc.vector.reciprocal(out=rz, in_=rz)

                prods = conv_pool.tile([128, GPB, NCLS, NCLS], F32, name="prods", tag="prods")
                for a in range(NCLS):
                    nc.vector.tensor_tensor(
                        out=prods[:, :, a, :],
                        in0=e_blk[:, 1, :, :],
                        in1=e_blk[:, 0, :, a : a + 1].to_broadcast(
                            [128, GPB, NCLS]
                        ),
                        op=mybir.AluOpType.mult,
                    )

                acc = out_pool.tile([128, GPB, NSUM], F32, name="acc", tag="acc")
                nc.vector.memset(acc, 0.0)
                for a in range(NCLS):
                    nc.vector.tensor_tensor(
                        out=acc[:, :, a : a + NCLS],
                        in0=acc[:, :, a : a + NCLS],
                        in1=prods[:, :, a, :],
                        op=mybir.AluOpType.add,
                    )
                nc.vector.tensor_tensor(
                    out=acc,
                    in0=acc,
                    in1=rz[:, :, None].to_broadcast([128, GPB, NSUM]),
                    op=mybir.AluOpType.mult,
                )
                nc.sync.dma_start(
                    out=out.ap()[blk * GPB * 128 : (blk + 1) * GPB * 128, :].rearrange(
                        "(g p) s -> p g s", p=128
                    ),
                    in_=acc,
                )

    nc.compile()
    return nc


_NC_CACHE = {}


def _get_nc():
    if "nc" not in _NC_CACHE:
        _NC_CACHE["nc"] = build_nc()
    return _NC_CACHE["nc"]


def _install_ntff_hook():
    """Shim antenv.axon_hooks (absent in this image) so trace=True can work."""
    try:
        import antenv

        if hasattr(antenv, "axon_hooks"):
            return
        from trn_agent_boot.trn_boot import _ntff_profile_via_ctypes

        mod = types.ModuleType("antenv.axon_hooks")
        holder = {"hook": _ntff_profile_via_ctypes("/opt/axon/libaxon_pjrt.so")}
        mod.set_axon_ntff_profile_hook = lambda h: holder.__setitem__("hook", h)
        mod.get_axon_ntff_profile_hook = lambda: holder["hook"]
        sys.modules["antenv.axon_hooks"] = mod
        antenv.axon_hooks = mod
    except Exception:
        pass


def kernel(x, W1_0, b1_0, W2_0, b2_0, W1_1, b1_1, W2_1, b2_1, _trace=False):
    x = np.asarray(x, dtype=np.float32)

    xf = x.reshape(2, B, F)
    weights = {}
    for n, (W1n, b1n, W2n, b2n) in enumerate(
        [(W1_0, b1_0, W2_0, b2_0), (W1_1, b1_1, W2_1, b2_1)]
    ):
        weights[f"w1_{n}"] = pad_rows(
            np.asarray(W1n, np.float32).astype(np.float16), np.float16
        )
        weights[f"w2_{n}"] = np.ascontiguousarray(np.asarray(W2n, np.float32)).astype(np.float16)
        weights[f"b1_{n}"] = np.ascontiguousarray(np.asarray(b1n, np.float32))
        weights[f"eb2_{n}"] = np.exp(np.asarray(b2n, np.float32))

    in_maps = []
    for c in range(NCORES):
        m = dict(weights)
        for n in range(2):
            shard = xf[n, c * BC : (c + 1) * BC, :]  # [BC, F]
            m[f"xt{n}"] = pad_rows(np.ascontiguousarray(shard.T), np.float32)
        in_maps.append(m)

    nc = _get_nc()
    if _trace:
        _install_ntff_hook()
    res = run_bass_kernel_spmd(nc, in_maps, list(range(NCORES)), trace=_trace)
    pieces = [res.results[c]["out"] for c in range(NCORES)]
    full = np.concatenate(pieces, axis=0).astype(np.float32)
    if _trace:
        return full, res
    return full


# revision 23
# speedup vs baseline: 1.1705x; 1.0434x over previous
"""Trainium2 Bass kernel for NetworksPlusCircuit.

Two MLPs (784->512->10, relu, softmax) over B=65536 samples each, then
P(sum=s) = sum_{a+c=s} p0[a]*p1[c]  -> [B, 19].

Sharding: pure data parallel over the batch across 8 NeuronCores.
Host-side prep: shard + transpose x to [784, B/8] per core so the device
DMA streams are contiguous; weights replicated (cast to bf16 on host).

Device-side per core (BC = 8192 samples per net):
  per round (512 samples of one net):
    - SWDGE cast-DMA loads xT tile [128f, 7fc, 512b] (fp32 HBM -> bf16 SBUF)
    - mm1: hT[j, b] += W1[f, j].T @ xT[f, b]   (PSUM, 4 j-chunks x 7 f-chunks)
    - bias+relu+cast -> SBUF bf16 (split across DVE and ACT)
    - mm2: l[b, 10]  += hT[j, b].T @ W2[j, 10] (PSUM, accumulate 4 j-chunks)
    - exp (ACT, PSUM -> SBUF block buffer), softmax denominator deferred
  per block (8 pair-rounds = 4096 sample pairs):
    - scale by exp(b2), Z sums, digit-sum convolution, normalize (DVE), DMA out
"""

import sys
import types

import numpy as np
import concourse.bass as bass
import concourse.bacc as bacc
import concourse.mybir as mybir
import concourse.tile as tile
from concourse.bass_utils import run_bass_kernel_spmd

F32 = mybir.dt.float32
MMDT = mybir.dt.float16  # matmul dtype: fp16 = bf16 speed, 4x less rounding error

NCORES = 8
B = 65536
BC = B // NCORES            # samples per core per net
F = 784                     # input features
HID = 512
NCLS = 10
NSUM = 19
FCH_FULL = F // 128         # 6 full feature chunks
FT = F - FCH_FULL * 128     # 16 tail features
FCH = FCH_FULL + 1          # 7 chunks
JCH = HID // 128            # 4 hidden chunks
# The 16 tail features are replicated at 4 32-row strips (rows 768+32k..+16)
# so the 4 per-j-chunk tail matmuls sit in distinct PE row-groups and run
# concurrently (row tiling).
FPAD = FCH * 128  # padded rows: 6*128 full chunks + 4 strips * 32 = 896
RN = 512                    # samples per round
NR = BC // RN               # 16 pair-rounds per core
NBLK = 8                    # conv blocks; NR % NBLK == 0
RPB = NR // NBLK            # pair-rounds per block
GPB = RPB * (RN // 128)     # 128-sample groups per block


def pad_rows(a, dtype):
    """[F, ...] -> [FPAD, ...] with tail rows replicated at 4 32-row strips."""
    out = np.zeros((FPAD,) + a.shape[1:], dtype=dtype)
    out[: FCH_FULL * 128] = a[: FCH_FULL * 128]
    for k in range(JCH):
        base = FCH_FULL * 128 + 32 * k
        out[base : base + FT] = a[FCH_FULL * 128 :]
    return out


def build_nc():
    nc = bacc.Bacc("TRN2", target_bir_lowering=False, debug=False, num_devices=NCORES)

    xt = [
        nc.dram_tensor(f"xt{n}", [FPAD, BC], F32, kind="ExternalInput")
        for n in range(2)
    ]
    w1 = [
        nc.dram_tensor(f"w1_{n}", [FPAD, HID], MMDT, kind="ExternalInput")
        for n in range(2)
    ]
    w2 = [
        nc.dram_tensor(f"w2_{n}", [HID, NCLS], MMDT, kind="ExternalInput")
        for n in range(2)
    ]
    b1 = [
        nc.dram_tensor(f"b1_{n}", [HID], F32, kind="ExternalInput") for n in range(2)
    ]
    eb2 = [
        nc.dram_tensor(f"eb2_{n}", [NCLS], F32, kind="ExternalInput") for n in range(2)
    ]
    out = nc.dram_tensor("out", [BC, NSUM], F32, kind="ExternalOutput")

    with tile.TileContext(nc) as tc:
        with (
            tc.tile_pool(name="consts", bufs=1) as consts,
            tc.tile_pool(name="xt", bufs=4) as xt_pool,
            tc.tile_pool(name="ht", bufs=2) as ht_pool,
            tc.tile_pool(name="eblk", bufs=3) as e_pool,
            tc.tile_pool(name="conv", bufs=1) as conv_pool,
            tc.tile_pool(name="outp", bufs=2) as out_pool,
            tc.tile_pool(name="hpsum", bufs=6, space="PSUM") as h_pool,
            tc.tile_pool(name="lpsum", bufs=2, space="PSUM") as l_pool,
        ):
            # ---- constants ----
            w1_sb = []
            w2_sb = []
            b1_sb = []
            eb2_sb = []
            for n in range(2):
                w = consts.tile([128, FCH, HID], MMDT, name=f"w1sb{n}", tag=f"w1sb{n}")
                nc.sync.dma_start(
                    out=w, in_=w1[n].ap().rearrange("(c p) j -> p c j", p=128)
                )
                w1_sb.append(w)

                w2t = consts.tile([128, JCH, NCLS], MMDT, name=f"w2sb{n}", tag=f"w2sb{n}")
                nc.sync.dma_start(
                    out=w2t, in_=w2[n].ap().rearrange("(c p) a -> p c a", p=128)
                )
                w2_sb.append(w2t)

                b1t = consts.tile([128, JCH], F32, name=f"b1sb{n}", tag=f"b1sb{n}")
                nc.gpsimd.dma_start(
                    out=b1t, in_=b1[n].ap().rearrange("(c p) -> p c", p=128)
                )
                b1_sb.append(b1t)

                ebt = consts.tile([128, NCLS], F32, name=f"eb2sb{n}", tag=f"eb2sb{n}")
                src = eb2[n].ap()
                nc.gpsimd.dma_start(
                    out=ebt,
                    in_=bass.AP(tensor=src.tensor, offset=0, ap=[[0, 128], [1, NCLS]]),
                )
                eb2_sb.append(ebt)

            # ---- main loop ----
            for blk in range(NBLK):
                # exp values for this block: [128, net, group, cls]
                e_blk = e_pool.tile([128, 2, GPB, NCLS], F32, name="eblk", tag="eblk")

                for rr in range(RPB):
                    r = blk * RPB + rr
                    for n in range(2):
                        xtile = xt_pool.tile([128, FCH, RN], MMDT, name="xt", tag="xt")
                        src = xt[n].ap()[:, r * RN : (r + 1) * RN]
                        # two DMAs: first chunks land earlier so mm1 starts
                        # sooner; also spreads descriptor generation
                        nc.gpsimd.dma_start(
                            out=xtile[:, 0:3, :],
                            in_=src[0 : 3 * 128, :].rearrange(
                                "(c p) b -> p c b", p=128
                            ),
                        )
                        nc.gpsimd.dma_start(
                            out=xtile[:, 3:FCH, :],
                            in_=src[3 * 128 : FCH * 128, :].rearrange(
                                "(c p) b -> p c b", p=128
                            ),
                        )

                        ph = [
                            h_pool.tile([128, RN], F32, name="ph", tag="ph")
                            for i in range(JCH)
                        ]
                        for fc in range(FCH_FULL):
                            for jc in range(JCH):
                                nc.tensor.matmul(
                                    out=ph[jc][:, :],
                                    lhsT=w1_sb[n][:, fc, jc * 128 : (jc + 1) * 128],
                                    rhs=xtile[:, fc, :],
                                    start=(fc == 0),
                                    stop=False,
                                )
                        # tail features: 4 concurrent row-tiled K=16 matmuls
                        for jc in range(JCH):
                            p0 = 32 * jc
                            nc.tensor.matmul(
                                out=ph[jc][:, :],
                                lhsT=w1_sb[n][
                                    p0 : p0 + FT, FCH_FULL, jc * 128 : (jc + 1) * 128
                                ],
                                rhs=xtile[p0 : p0 + FT, FCH_FULL, :],
                                start=False,
                                stop=True,
                                tile_position=(p0, 0),
                            )

                        ht = ht_pool.tile([128, JCH, RN], MMDT, name="ht", tag="ht")
                        for jc in range(JCH):
                            # all on ACT: DVE must stay free for the conv
                            # blocks or PSUM drains stall and starve the PE
                            nc.scalar.activation(
                                out=ht[:, jc, :],
                                in_=ph[jc][:, :],
                                func=mybir.ActivationFunctionType.Relu,
                                bias=b1_sb[n][:, jc : jc + 1],
                                scale=1.0,
                            )

                        pl = l_pool.tile([128, RN // 128, NCLS], F32, name="pl", tag="pl")
                        for bc in range(RN // 128):
                            for jc in range(JCH):
                                nc.tensor.matmul(
                                    out=pl[:, bc, :],
                                    lhsT=ht[:, jc, bc * 128 : (bc + 1) * 128],
                                    rhs=w2_sb[n][:, jc, :],
                                    start=(jc == 0),
                                    stop=(jc == JCH - 1),
                                )

                        nc.scalar.activation(
                            out=e_blk[:, n, rr * 4 : (rr + 1) * 4, :],
                            in_=pl[:, :, :],
                            func=mybir.ActivationFunctionType.Exp,
                        )

                # ---- conv block over GPB groups ----
                # scale by exp(b2) (free-broadcast over groups)
                for n in range(2):
                    nc.vector.tensor_tensor(
                        out=e_blk[:, n, :, :],
                        in0=e_blk[:, n, :, :],
                        in1=eb2_sb[n][:, None, :].to_broadcast([128, GPB, NCLS]),
                        op=mybir.AluOpType.mult,
                    )

                z = conv_pool.tile([128, 2, GPB], F32, name="z", tag="z")
                for n in range(2):
                    nc.vector.reduce_sum(
                        out=z[:, n, :],
                        in_=e_blk[:, n, :, :],
                        axis=mybir.AxisListType.X,
                    )
                rz = conv_pool.tile([128, GPB], F32, name="rz", tag="rz")
                nc.vector.tensor_tensor(
                    out=rz, in0=z[:, 0, :], in1=z[:, 1, :], op=mybir.AluOpType.mult
                )
                nc.vector.reciprocal(out=rz, in_=rz)

                prods = conv_pool.tile([128, GPB, NCLS, NCLS], F32, name="prods", tag="prods")
                for a in range(NCLS):
                    nc.vector.tensor_tensor(
                        out=prods[:, :, a, :],
                        in0=e_blk[:, 1, :, :],
                        in1=e_blk[:, 0, :, a : a + 1].to_broadcast(
                            [128, GPB, NCLS]
                        ),
                        op=mybir.AluOpType.mult,
                    )

                acc = out_pool.tile([128, GPB, NSUM], F32, name="acc", tag="acc")
                nc.vector.memset(acc, 0.0)
                for a in range(NCLS):
                    nc.vector.tensor_tensor(
                        out=acc[:, :, a : a + NCLS],
                        in0=acc[:, :, a : a + NCLS],
                        in1=prods[:, :, a, :],
                        op=mybir.AluOpType.add,
                    )
                nc.vector.tensor_tensor(
                    out=acc,
                    in0=acc,
                    in1=rz[:, :, None].to_broadcast([128, GPB, NSUM]),
                    op=mybir.AluOpType.mult,
                )
                nc.sync.dma_start(
                    out=out.ap()[blk * GPB * 128 : (blk + 1) * GPB * 128, :].rearrange(
                        "(g p) s -> p g s", p=128
                    ),
                    in_=acc,
                )

    nc.compile()
    return nc


_NC_CACHE = {}


def _get_nc():
    if "nc" not in _NC_CACHE:
        _NC_CACHE["nc"] = build_nc()
    return _NC_CACHE["nc"]


def _install_ntff_hook():
    """Shim antenv.axon_hooks (absent in this image) so trace=True can work."""
    try:
        import antenv

        if hasattr(antenv, "axon_hooks"):
            return
        from trn_agent_boot.trn_boot import _ntff_profile_via_ctypes

        mod = types.ModuleType("antenv.axon_hooks")
        holder = {"hook": _ntff_profile_via_ctypes("/opt/axon/libaxon_pjrt.so")}
        mod.set_axon_ntff_profile_hook = lambda h: holder.__setitem__("hook", h)
        mod.get_axon_ntff_profile_hook = lambda: holder["hook"]
        sys.modules["antenv.axon_hooks"] = mod
        antenv.axon_hooks = mod
    except Exception:
        pass


def kernel(x, W1_0, b1_0, W2_0, b2_0, W1_1, b1_1, W2_1, b2_1, _trace=False):
    x = np.asarray(x, dtype=np.float32)

    xf = x.reshape(2, B, F)
    weights = {}
    for n, (W1n, b1n, W2n, b2n) in enumerate(
        [(W1_0, b1_0, W2_0, b2_0), (W1_1, b1_1, W2_1, b2_1)]
    ):
        weights[f"w1_{n}"] = pad_rows(
            np.asarray(W1n, np.float32).astype(np.float16), np.float16
        )
        weights[f"w2_{n}"] = np.ascontiguousarray(np.asarray(W2n, np.float32)).astype(np.float16)
        weights[f"b1_{n}"] = np.ascontiguousarray(np.asarray(b1n, np.float32))
        weights[f"eb2_{n}"] = np.exp(np.asarray(b2n, np.float32))

    in_maps = []
    for c in range(NCORES):
        m = dict(weights)
        for n in range(2):
            shard = xf[n, c * BC : (c + 1) * BC, :]  # [BC, F]
            m[f"xt{n}"] = pad_rows(np.ascontiguousarray(shard.T), np.float32)
        in_maps.append(m)

    nc = _get_nc()
    if _trace:
        _install_ntff_hook()
    res = run_bass_kernel_spmd(nc, in_maps, list(range(NCORES)), trace=_trace)
    pieces = [res.results[c]["out"] for c in range(NCORES)]
    full = np.concatenate(pieces, axis=0).astype(np.float32)
    if _trace:
        return full, res
    return full
